# revision 5
# baseline (speedup 1.0000x reference)
import numpy as np
import ml_dtypes

import concourse.bacc as bacc
import concourse.bass as bass
import concourse.mybir as mybir
import concourse.tile as tile
from concourse import bass_utils

bf16 = ml_dtypes.bfloat16

B, N, D = 4, 2048, 1024
NQ, NK = 1024, 2048
FP32 = mybir.dt.float32
BF16 = mybir.dt.bfloat16
F32R = mybir.dt.float32r
EXP = mybir.ActivationFunctionType.Exp
SQRT = mybir.ActivationFunctionType.Sqrt

LAST_EXEC_NS = None
_NC = None


def _broadcast_ap(dram_ap, parts):
    return bass.AP(
        tensor=dram_ap.tensor,
        offset=dram_ap.offset,
        ap=[[0, parts], dram_ap.ap[-1]],
    )


def _build():
    nc = bacc.Bacc(None, target_bir_lowering=False)
    qT = nc.dram_tensor("qT", [D, NQ], BF16, kind="ExternalInput")
    qn = nc.dram_tensor("qn", [NQ, D], FP32, kind="ExternalInput")
    kT = nc.dram_tensor("kT", [D, NK], BF16, kind="ExternalInput")
    vT = nc.dram_tensor("vT", [D, NK], BF16, kind="ExternalInput")
    wq = nc.dram_tensor("wq", [D, D], BF16, kind="ExternalInput")
    wk = nc.dram_tensor("wk", [D, D], BF16, kind="ExternalInput")
    wv = nc.dram_tensor("wv", [D, D], BF16, kind="ExternalInput")
    wo = nc.dram_tensor("wo", [D, D], BF16, kind="ExternalInput")
    gamma = nc.dram_tensor("gamma", [1, D], FP32, kind="ExternalInput")
    beta = nc.dram_tensor("beta", [1, D], FP32, kind="ExternalInput")
    out = nc.dram_tensor("out", [NQ, D], FP32, kind="ExternalOutput")

    with tile.TileContext(nc) as tc:
        with (
            tc.tile_pool(name="perm", bufs=1) as perm,
            tc.tile_pool(name="ps", bufs=1, space="PSUM") as ps,
        ):
            gamma_t = perm.tile([128, D], FP32)
            beta_t = perm.tile([128, D], FP32)
            nc.gpsimd.dma_start(out=gamma_t, in_=_broadcast_ap(gamma[0:1, :], 128))
            nc.gpsimd.dma_start(out=beta_t, in_=_broadcast_ap(beta[0:1, :], 128))
            eps_t = perm.tile([128, 1], FP32)
            nc.vector.memset(eps_t, 1e-5)
            ones_f = perm.tile([128, 64], FP32)
            nc.vector.memset(ones_f, 1.0)
            ones_t = perm.tile([128, 64], F32R)
            nc.vector.tensor_copy(ones_t, ones_f)

            wq_t = perm.tile([128, 8, D], BF16)
            wo_t = perm.tile([128, 8, D], BF16)
            nc.sync.dma_start(wq_t, wq[:, :].rearrange("(dt p) f -> p dt f", p=128))
            nc.sync.dma_start(wo_t, wo[:, :].rearrange("(dt p) f -> p dt f", p=128))

            # ksT[jt][p, m] = ks[m, jt*128+p]; vsp[kt][p, 65h:65h+64] = vs[kt*128+p, 64h:64h+64], col 65h+64 = 1
            ksT = [perm.tile([128, NK], BF16, tag=f"ks{j}", name=f"ks{j}") for j in range(8)]
            vsp = [perm.tile([128, 16 * 65], BF16, tag=f"vsp{t}", name=f"vsp{t}") for t in range(16)]

            with tc.tile_pool(name="kv", bufs=1) as kv:
                wkv_t = kv.tile([128, 8, D], BF16, tag="wkv")
                nc.sync.dma_start(wkv_t, wv[:, :].rearrange("(dt p) f -> p dt f", p=128))
                for rc in range(8):
                    vstage = kv.tile([128, 8, 256], BF16, tag="stage", bufs=2)
                    nc.sync.dma_start(
                        vstage,
                        vT[:, rc * 256:(rc + 1) * 256].rearrange("(dt p) n -> p dt n", p=128),
                    )
                    for rt in range(2):
                        kt_i = rc * 2 + rt
                        for jc in range(2):
                            pp = ps.tile([128, 512], FP32, tag="pp", bufs=2)
                            for dt_i in range(8):
                                nc.tensor.matmul(
                                    pp,
                                    vstage[:, dt_i, rt * 128:(rt + 1) * 128],
                                    wkv_t[:, dt_i, jc * 512:(jc + 1) * 512],
                                    start=(dt_i == 0),
                                    stop=(dt_i == 7),
                                )
                            nc.vector.tensor_copy(
                                vsp[kt_i].rearrange("p (h c) -> p h c", h=16)[:, jc * 8:(jc + 1) * 8, 0:64],
                                pp.rearrange("p (h c) -> p h c", h=8),
                            )
                        nc.vector.memset(
                            vsp[kt_i].rearrange("p (h c) -> p h c", h=16)[:, :, 64:65], 1.0
                        )

                nc.sync.dma_start(wkv_t, wk[:, :].rearrange("(dt p) f -> p dt f", p=128))
                for kc in range(8):
                    kstage = kv.tile([128, 8, 256], BF16, tag="stage", bufs=2)
                    nc.sync.dma_start(
                        kstage,
                        kT[:, kc * 256:(kc + 1) * 256].rearrange("(dt p) n -> p dt n", p=128),
                    )
                    for jt in range(8):
                        pp = ps.tile([128, 256], FP32, tag="pp", bufs=2)
                        for dt_i in range(8):
                            nc.tensor.matmul(
                                pp,
                                wkv_t[:, dt_i, jt * 128:(jt + 1) * 128],
                                kstage[:, dt_i, :],
                                start=(dt_i == 0),
                                stop=(dt_i == 7),
                            )
                        nc.vector.tensor_copy(ksT[jt][:, kc * 256:(kc + 1) * 256], pp)

            with tc.tile_pool(name="qp", bufs=1) as qp:
                for qc in range(2):
                    qstage = qp.tile([128, 8, 512], BF16, tag="qstage")
                    nc.sync.dma_start(
                        qstage,
                        qT[:, qc * 512:(qc + 1) * 512].rearrange("(dt p) n -> p dt n", p=128),
                    )
                    qs_t = [qp.tile([128, 512], BF16, tag=f"qs{j}", name=f"qs{j}") for j in range(8)]
                    for jt in range(8):
                        pp = ps.tile([128, 512], FP32, tag="pp", bufs=2)
                        for dt_i in range(8):
                            nc.tensor.matmul(
                                pp,
                                wq_t[:, dt_i, jt * 128:(jt + 1) * 128],
                                qstage[:, dt_i, :],
                                start=(dt_i == 0),
                                stop=(dt_i == 7),
                            )
                        nc.vector.tensor_copy(qs_t[jt], pp)

                    at_t = [qp.tile([128, 512], BF16, tag=f"at{j}", name=f"at{j}", bufs=2) for j in range(8)]
                    pending = [None]

                    def emit_norm(p, at_t=at_t):
                        ues_, uos_, hp_ = p
                        rd1 = qp.tile([128, 512], F32R, tag="rd", bufs=2)
                        with nc.allow_low_precision(reason="f32r recip, ~1.5e-4 rel"):
                            nc.vector.reciprocal(rd1[64:65, :], ues_[64:65, :])
                        bce = ps.tile([128, 512], FP32, tag="pp", bufs=2)
                        nc.tensor.matmul(
                            bce[0:64, :], ones_t[64:65, 0:64], rd1[64:65, :],
                            start=True, stop=True, tile_position=(64, 0),
                        )
                        nc.vector.tensor_mul(at_t[hp_][0:64, :], ues_[0:64, :], bce[0:64, :])
                        rd2 = qp.tile([128, 512], F32R, tag="rd", bufs=2)
                        with nc.allow_low_precision(reason="f32r recip, ~1.5e-4 rel"):
                            nc.vector.reciprocal(rd2[64:65, :], uos_[64:65, :])
                        bco = ps.tile([128, 512], FP32, tag="pp", bufs=2)
                        nc.tensor.matmul(
                            bco[0:64, :], ones_t[64:65, 0:64], rd2[64:65, :],
                            start=True, stop=True, tile_position=(64, 0),
                        )
                        nc.vector.tensor_mul(at_t[hp_][64:128, :], uos_[0:64, :], bco[0:64, :])

                    for hp in range(8):
                        ue = ps.tile([128, 512], FP32, tag="ue")
                        uo = ps.tile([128, 512], FP32, tag="uo")
                        Es = {}
                        for kt in range(16):
                            se = ps.tile([128, 512], FP32, tag="se", bufs=2)
                            so = ps.tile([128, 512], FP32, tag="so", bufs=2)
                            nc.tensor.matmul(
                                se, ksT[hp][0:64, kt * 128:(kt + 1) * 128],
                                qs_t[hp][0:64, :], start=True, stop=True,
                            )
                            nc.tensor.matmul(
                                so, ksT[hp][64:128, kt * 128:(kt + 1) * 128],
                                qs_t[hp][64:128, :], start=True, stop=True,
                                tile_position=(64, 0),
                            )
                            if kt > 0:
                                Ep, E2p = Es.pop(kt - 1)
                                nc.tensor.matmul(
                                    ue[0:65, :], vsp[kt - 1][:, 130 * hp:130 * hp + 65], Ep,
                                    start=(kt == 1), stop=False, skip_group_check=True,
                                )
                                nc.tensor.matmul(
                                    uo[0:65, :], vsp[kt - 1][:, 130 * hp + 65:130 * hp + 130], E2p,
                                    start=(kt == 1), stop=False, skip_group_check=True,
                                )
                            if kt == 2 and pending[0] is not None:
                                emit_norm(pending[0])
                                pending[0] = None
                            E = qp.tile([128, 512], BF16, tag="E", bufs=4)
                            nc.scalar.activation(E, se, func=EXP, bias=0.0, scale=0.125)
                            E2 = qp.tile([128, 512], BF16, tag="E", bufs=4)
                            nc.scalar.activation(E2, so, func=EXP, bias=0.0, scale=0.125)
                            Es[kt] = (E, E2)
                        Ep, E2p = Es.pop(15)
                        nc.tensor.matmul(
                            ue[0:65, :], vsp[15][:, 130 * hp:130 * hp + 65], Ep,
                            start=False, stop=True, skip_group_check=True,
                        )
                        nc.tensor.matmul(
                            uo[0:65, :], vsp[15][:, 130 * hp + 65:130 * hp + 130], E2p,
                            start=False, stop=True, skip_group_check=True,
                        )
                        ues = qp.tile([128, 512], FP32, tag="ues", bufs=2)
                        uos = qp.tile([128, 512], FP32, tag="uos", bufs=2)
                        nc.vector.tensor_copy(ues[0:65, :], ue[0:65, :])
                        nc.vector.tensor_copy(uos[0:65, :], uo[0:65, :])
                        pending[0] = (ues, uos, hp)
                    emit_norm(pending[0])
                    pending[0] = None

                    for rt in range(4):
                        row0 = qc * 512 + rt * 128
                        outf = qp.tile([128, D], FP32, tag="outf", bufs=2)
                        nc.sync.dma_start(outf, qn[row0:row0 + 128, :])
                        for oc in range(2):
                            po = ps.tile([128, 512], FP32, tag="pp", bufs=2)
                            for it in range(8):
                                nc.tensor.matmul(
                                    po, at_t[it][:, rt * 128:(rt + 1) * 128],
                                    wo_t[:, it, oc * 512:(oc + 1) * 512],
                                    start=(it == 0), stop=(it == 7),
                                )
                            nc.vector.tensor_add(
                                out=outf[:, oc * 512:(oc + 1) * 512],
                                in0=outf[:, oc * 512:(oc + 1) * 512], in1=po,
                            )
                        bst = qp.tile([128, 2, 6], FP32, tag="bst", bufs=2)
                        mv = qp.tile([128, 2], FP32, tag="mv", bufs=2)
                        for sg in range(2):
                            nc.vector.bn_stats(out=bst[:, sg, :], in_=outf[:, sg * 512:(sg + 1) * 512])
                        nc.vector.bn_aggr(out=mv, in_=bst)
                        nc.scalar.activation(
                            out=mv[:, 1:2], in_=mv[:, 1:2], func=SQRT,
                            bias=eps_t[:, :], scale=1.0,
                        )
                        nc.vector.reciprocal(mv[:, 1:2], mv[:, 1:2])
                        y = qp.tile([128, D], FP32, tag="y")
                        nc.vector.tensor_scalar(
                            out=y, in0=outf, scalar1=mv[:, 0:1], scalar2=mv[:, 1:2],
                            op0=mybir.AluOpType.subtract, op1=mybir.AluOpType.mult,
                        )
                        nc.vector.tensor_mul(y, y, gamma_t)
                        nc.vector.tensor_add(out=y, in0=y, in1=beta_t)
                        nc.sync.dma_start(out[row0:row0 + 128, :], y)
    nc.finalize()
    return nc


def kernel(q, k, v, Wq, Wk, Wv, Wo, gamma, beta, _trace=False):
    global _NC, LAST_EXEC_NS
    if _NC is None:
        _NC = _build()
    wqh = Wq.T.astype(bf16)
    wkh = Wk.T.astype(bf16)
    wvh = Wv.T.astype(bf16)
    woh = Wo.T.astype(bf16)
    g = np.ascontiguousarray(np.asarray(gamma, dtype=np.float32).reshape(1, D))
    bt = np.ascontiguousarray(np.asarray(beta, dtype=np.float32).reshape(1, D))
    in_maps = []
    for c in range(8):
        b, hh = divmod(c, 2)
        qb = q[b, hh * NQ:(hh + 1) * NQ, :]
        in_maps.append({
            "qT": qb.T.astype(bf16),
            "qn": np.ascontiguousarray(qb, dtype=np.float32),
            "kT": k[b].T.astype(bf16),
            "vT": v[b].T.astype(bf16),
            "wq": wqh, "wk": wkh, "wv": wvh, "wo": woh,
            "gamma": g, "beta": bt,
        })
    res = bass_utils.run_bass_kernel_spmd(_NC, in_maps, list(range(8)), trace=_trace)
    LAST_EXEC_NS = getattr(res, "exec_time_ns", None)
    outp = np.empty((B, N, D), np.float32)
    for c in range(8):
        b, hh = divmod(c, 2)
        outp[b, hh * NQ:(hh + 1) * NQ, :] = res.results[c]["out"]
    return outp


# revision 8
# speedup vs baseline: 1.2900x; 1.2900x over previous
import numpy as np
import ml_dtypes

import concourse.bacc as bacc
import concourse.bass as bass
import concourse.mybir as mybir
import concourse.tile as tile
from concourse import bass_utils

bf16 = ml_dtypes.bfloat16

B, N, D = 4, 2048, 1024
NQ, NK = 1024, 2048
FP32 = mybir.dt.float32
BF16 = mybir.dt.bfloat16
F32R = mybir.dt.float32r
EXP = mybir.ActivationFunctionType.Exp
SQRT = mybir.ActivationFunctionType.Sqrt

LAST_EXEC_NS = None
_NC = None


def _broadcast_ap(dram_ap, parts):
    return bass.AP(
        tensor=dram_ap.tensor,
        offset=dram_ap.offset,
        ap=[[0, parts], dram_ap.ap[-1]],
    )


def _build():
    nc = bacc.Bacc(None, target_bir_lowering=False)
    qT = nc.dram_tensor("qT", [D, NQ], BF16, kind="ExternalInput")
    qn = nc.dram_tensor("qn", [NQ, D], FP32, kind="ExternalInput")
    kT = nc.dram_tensor("kT", [D, NK], BF16, kind="ExternalInput")
    vT = nc.dram_tensor("vT", [D, NK], BF16, kind="ExternalInput")
    wq = nc.dram_tensor("wq", [D, D], BF16, kind="ExternalInput")
    wk = nc.dram_tensor("wk", [D, D], BF16, kind="ExternalInput")
    wv = nc.dram_tensor("wv", [D, D], BF16, kind="ExternalInput")
    wo = nc.dram_tensor("wo", [D, D], BF16, kind="ExternalInput")
    gamma = nc.dram_tensor("gamma", [1, D], FP32, kind="ExternalInput")
    beta = nc.dram_tensor("beta", [1, D], FP32, kind="ExternalInput")
    out = nc.dram_tensor("out", [NQ, D], FP32, kind="ExternalOutput")

    with tile.TileContext(nc) as tc:
        with (
            tc.tile_pool(name="perm", bufs=1) as perm,
            tc.tile_pool(name="ps", bufs=1, space="PSUM") as ps,
        ):
            gamma_t = perm.tile([128, D], FP32)
            beta_t = perm.tile([128, D], FP32)
            nc.gpsimd.dma_start(out=gamma_t, in_=_broadcast_ap(gamma[0:1, :], 128))
            nc.gpsimd.dma_start(out=beta_t, in_=_broadcast_ap(beta[0:1, :], 128))
            eps_t = perm.tile([128, 1], FP32)
            nc.vector.memset(eps_t, 1e-5)
            ones_f = perm.tile([128, 64], FP32)
            nc.vector.memset(ones_f, 1.0)
            ones_t = perm.tile([128, 64], F32R)
            nc.vector.tensor_copy(ones_t, ones_f)

            wq_t = perm.tile([128, 8, D], BF16)
            wo_t = perm.tile([128, 8, D], BF16)
            nc.sync.dma_start(wq_t, wq[:, :].rearrange("(dt p) f -> p dt f", p=128))
            nc.sync.dma_start(wo_t, wo[:, :].rearrange("(dt p) f -> p dt f", p=128))

            # ksT[jt][p, m] = ks[m, jt*128+p]; vsp[kt][p, 65h:65h+64] = vs[kt*128+p, 64h:64h+64], col 65h+64 = 1
            ksT = [perm.tile([128, NK], BF16, tag=f"ks{j}", name=f"ks{j}") for j in range(8)]
            vsp = [perm.tile([128, 16 * 65], BF16, tag=f"vsp{t}", name=f"vsp{t}") for t in range(16)]

            with tc.tile_pool(name="kv", bufs=1) as kv:
                wkv_t = kv.tile([128, 8, D], BF16, tag="wkv")
                nc.sync.dma_start(wkv_t, wv[:, :].rearrange("(dt p) f -> p dt f", p=128))
                for rc in range(8):
                    vstage = kv.tile([128, 8, 256], BF16, tag="stage", bufs=2)
                    nc.sync.dma_start(
                        vstage,
                        vT[:, rc * 256:(rc + 1) * 256].rearrange("(dt p) n -> p dt n", p=128),
                    )
                    for rt in range(2):
                        kt_i = rc * 2 + rt
                        for jc in range(2):
                            pp = ps.tile([128, 512], FP32, tag="pp", bufs=2)
                            for dt_i in range(8):
                                nc.tensor.matmul(
                                    pp,
                                    vstage[:, dt_i, rt * 128:(rt + 1) * 128],
                                    wkv_t[:, dt_i, jc * 512:(jc + 1) * 512],
                                    start=(dt_i == 0),
                                    stop=(dt_i == 7),
                                )
                            nc.vector.tensor_copy(
                                vsp[kt_i].rearrange("p (h c) -> p h c", h=16)[:, jc * 8:(jc + 1) * 8, 0:64],
                                pp.rearrange("p (h c) -> p h c", h=8),
                            )
                        nc.vector.memset(
                            vsp[kt_i].rearrange("p (h c) -> p h c", h=16)[:, :, 64:65], 1.0
                        )

                nc.sync.dma_start(wkv_t, wk[:, :].rearrange("(dt p) f -> p dt f", p=128))
                for kc in range(4):
                    kstage = kv.tile([128, 8, 512], BF16, tag="kstage", bufs=2)
                    nc.sync.dma_start(
                        kstage,
                        kT[:, kc * 512:(kc + 1) * 512].rearrange("(dt p) n -> p dt n", p=128),
                    )
                    for jt in range(8):
                        pp = ps.tile([128, 512], FP32, tag="pp", bufs=2)
                        for dt_i in range(8):
                            nc.tensor.matmul(
                                pp,
                                wkv_t[:, dt_i, jt * 128:(jt + 1) * 128],
                                kstage[:, dt_i, :],
                                start=(dt_i == 0),
                                stop=(dt_i == 7),
                            )
                        nc.vector.tensor_copy(ksT[jt][:, kc * 512:(kc + 1) * 512], pp)

            with tc.tile_pool(name="qp", bufs=1) as qp:
                for qc in range(2):
                    qstage = qp.tile([128, 8, 512], BF16, tag="qstage")
                    nc.sync.dma_start(
                        qstage,
                        qT[:, qc * 512:(qc + 1) * 512].rearrange("(dt p) n -> p dt n", p=128),
                    )
                    qs_t = [qp.tile([128, 512], BF16, tag=f"qs{j}", name=f"qs{j}") for j in range(8)]
                    for jt in range(8):
                        pp = ps.tile([128, 512], FP32, tag="pp", bufs=2)
                        for dt_i in range(8):
                            nc.tensor.matmul(
                                pp,
                                wq_t[:, dt_i, jt * 128:(jt + 1) * 128],
                                qstage[:, dt_i, :],
                                start=(dt_i == 0),
                                stop=(dt_i == 7),
                            )
                        nc.vector.tensor_copy(qs_t[jt], pp)

                    at_t = [qp.tile([128, 512], BF16, tag=f"at{j}", name=f"at{j}") for j in range(8)]
                    pending = [None]

                    def emit_norm(p, at_t=at_t):
                        ues_, uos_, hp_ = p
                        rd1 = qp.tile([128, 512], F32R, tag="rd", bufs=2)
                        with nc.allow_low_precision(reason="f32r recip, ~1.5e-4 rel"):
                            nc.vector.reciprocal(rd1[64:65, :], ues_[64:65, :])
                        bce = ps.tile([128, 512], FP32, tag="pp", bufs=2)
                        nc.tensor.matmul(
                            bce[0:64, :], ones_t[64:65, 0:64], rd1[64:65, :],
                            start=True, stop=True, tile_position=(64, 0),
                        )
                        nc.vector.tensor_mul(at_t[hp_][0:64, :], ues_[0:64, :], bce[0:64, :])
                        rd2 = qp.tile([128, 512], F32R, tag="rd", bufs=2)
                        with nc.allow_low_precision(reason="f32r recip, ~1.5e-4 rel"):
                            nc.vector.reciprocal(rd2[64:65, :], uos_[64:65, :])
                        bco = ps.tile([128, 512], FP32, tag="pp", bufs=2)
                        nc.tensor.matmul(
                            bco[0:64, :], ones_t[64:65, 0:64], rd2[64:65, :],
                            start=True, stop=True, tile_position=(64, 0),
                        )
                        nc.vector.tensor_mul(at_t[hp_][64:128, :], uos_[0:64, :], bco[0:64, :])

                    for hp in range(8):
                        ue = ps.tile([128, 512], FP32, tag="ue")
                        uo = ps.tile([128, 512], FP32, tag="uo")
                        Es = {}
                        for kt in range(16):
                            sc = ps.tile([128, 1024], FP32, tag="sc", bufs=2)
                            nc.tensor.matmul(
                                sc[:, 0:512], ksT[hp][0:64, kt * 128:(kt + 1) * 128],
                                qs_t[hp][0:64, :], start=True, stop=True,
                                skip_group_check=True,
                            )
                            nc.tensor.matmul(
                                sc[:, 512:1024], ksT[hp][64:128, kt * 128:(kt + 1) * 128],
                                qs_t[hp][64:128, :], start=True, stop=True,
                                tile_position=(64, 0), skip_group_check=True,
                            )
                            if kt > 0:
                                Ep = Es.pop(kt - 1)
                                nc.tensor.matmul(
                                    ue[0:65, :], vsp[kt - 1][:, 130 * hp:130 * hp + 65],
                                    Ep[:, 0:512],
                                    start=(kt == 1), stop=False, skip_group_check=True,
                                )
                                nc.tensor.matmul(
                                    uo[0:65, :], vsp[kt - 1][:, 130 * hp + 65:130 * hp + 130],
                                    Ep[:, 512:1024],
                                    start=(kt == 1), stop=False, skip_group_check=True,
                                )
                            if kt == 2 and pending[0] is not None:
                                emit_norm(pending[0])
                                pending[0] = None
                            E = qp.tile([128, 1024], BF16, tag="E", bufs=3)
                            nc.scalar.activation(E, sc, func=EXP, bias=0.0, scale=0.125)
                            Es[kt] = E
                        Ep = Es.pop(15)
                        nc.tensor.matmul(
                            ue[0:65, :], vsp[15][:, 130 * hp:130 * hp + 65], Ep[:, 0:512],
                            start=False, stop=True, skip_group_check=True,
                        )
                        nc.tensor.matmul(
                            uo[0:65, :], vsp[15][:, 130 * hp + 65:130 * hp + 130],
                            Ep[:, 512:1024],
                            start=False, stop=True, skip_group_check=True,
                        )
                        ues = qp.tile([128, 512], FP32, tag="ues", bufs=2)
                        uos = qp.tile([128, 512], FP32, tag="uos", bufs=2)
                        nc.vector.tensor_copy(ues[0:65, :], ue[0:65, :])
                        nc.vector.tensor_copy(uos[0:65, :], uo[0:65, :])
                        pending[0] = (ues, uos, hp)
                    emit_norm(pending[0])
                    pending[0] = None

                    for rt in range(4):
                        row0 = qc * 512 + rt * 128
                        outf = qp.tile([128, D], FP32, tag="outf", bufs=2)
                        nc.sync.dma_start(outf, qn[row0:row0 + 128, :])
                        for oc in range(2):
                            po = ps.tile([128, 512], FP32, tag="pp", bufs=2)
                            for it in range(8):
                                nc.tensor.matmul(
                                    po, at_t[it][:, rt * 128:(rt + 1) * 128],
                                    wo_t[:, it, oc * 512:(oc + 1) * 512],
                                    start=(it == 0), stop=(it == 7),
                                )
                            nc.vector.tensor_add(
                                out=outf[:, oc * 512:(oc + 1) * 512],
                                in0=outf[:, oc * 512:(oc + 1) * 512], in1=po,
                            )
                        bst = qp.tile([128, 2, 6], FP32, tag="bst", bufs=2)
                        mv = qp.tile([128, 2], FP32, tag="mv", bufs=2)
                        for sg in range(2):
                            nc.vector.bn_stats(out=bst[:, sg, :], in_=outf[:, sg * 512:(sg + 1) * 512])
                        nc.vector.bn_aggr(out=mv, in_=bst)
                        nc.scalar.activation(
                            out=mv[:, 1:2], in_=mv[:, 1:2], func=SQRT,
                            bias=eps_t[:, :], scale=1.0,
                        )
                        nc.vector.reciprocal(mv[:, 1:2], mv[:, 1:2])
                        y = qp.tile([128, D], FP32, tag="y")
                        nc.vector.tensor_scalar(
                            out=y, in0=outf, scalar1=mv[:, 0:1], scalar2=mv[:, 1:2],
                            op0=mybir.AluOpType.subtract, op1=mybir.AluOpType.mult,
                        )
                        nc.vector.tensor_mul(y, y, gamma_t)
                        nc.vector.tensor_add(out=y, in0=y, in1=beta_t)
                        nc.sync.dma_start(out[row0:row0 + 128, :], y)
    nc.finalize()
    return nc


def kernel(q, k, v, Wq, Wk, Wv, Wo, gamma, beta, _trace=False):
    global _NC, LAST_EXEC_NS
    if _NC is None:
        _NC = _build()
    wqh = Wq.T.astype(bf16)
    wkh = Wk.T.astype(bf16)
    wvh = Wv.T.astype(bf16)
    woh = Wo.T.astype(bf16)
    g = np.ascontiguousarray(np.asarray(gamma, dtype=np.float32).reshape(1, D))
    bt = np.ascontiguousarray(np.asarray(beta, dtype=np.float32).reshape(1, D))
    in_maps = []
    for c in range(8):
        b, hh = divmod(c, 2)
        qb = q[b, hh * NQ:(hh + 1) * NQ, :]
        in_maps.append({
            "qT": qb.T.astype(bf16),
            "qn": np.ascontiguousarray(qb, dtype=np.float32),
            "kT": k[b].T.astype(bf16),
            "vT": v[b].T.astype(bf16),
            "wq": wqh, "wk": wkh, "wv": wvh, "wo": woh,
            "gamma": g, "beta": bt,
        })
    res = bass_utils.run_bass_kernel_spmd(_NC, in_maps, list(range(8)), trace=_trace)
    LAST_EXEC_NS = getattr(res, "exec_time_ns", None)
    outp = np.empty((B, N, D), np.float32)
    for c in range(8):
        b, hh = divmod(c, 2)
        outp[b, hh * NQ:(hh + 1) * NQ, :] = res.results[c]["out"]
    return outp


# revision 9
# speedup vs baseline: 1.3124x; 1.0173x over previous
import numpy as np
import ml_dtypes

import concourse.bacc as bacc
import concourse.bass as bass
import concourse.mybir as mybir
import concourse.tile as tile
from concourse import bass_utils

bf16 = ml_dtypes.bfloat16

B, N, D = 4, 2048, 1024
NQ, NK = 1024, 2048
FP32 = mybir.dt.float32
BF16 = mybir.dt.bfloat16
F32R = mybir.dt.float32r
EXP = mybir.ActivationFunctionType.Exp
SQRT = mybir.ActivationFunctionType.Sqrt

LAST_EXEC_NS = None
_NC = None


def _broadcast_ap(dram_ap, parts):
    return bass.AP(
        tensor=dram_ap.tensor,
        offset=dram_ap.offset,
        ap=[[0, parts], dram_ap.ap[-1]],
    )


def _build():
    nc = bacc.Bacc(None, target_bir_lowering=False)
    qT = nc.dram_tensor("qT", [D, NQ], BF16, kind="ExternalInput")
    qn = nc.dram_tensor("qn", [NQ, D], FP32, kind="ExternalInput")
    kT = nc.dram_tensor("kT", [D, NK], BF16, kind="ExternalInput")
    vT = nc.dram_tensor("vT", [D, NK], BF16, kind="ExternalInput")
    wq = nc.dram_tensor("wq", [D, D], BF16, kind="ExternalInput")
    wk = nc.dram_tensor("wk", [D, D], BF16, kind="ExternalInput")
    wv = nc.dram_tensor("wv", [D, D], BF16, kind="ExternalInput")
    wo = nc.dram_tensor("wo", [D, D], BF16, kind="ExternalInput")
    gamma = nc.dram_tensor("gamma", [1, D], FP32, kind="ExternalInput")
    beta = nc.dram_tensor("beta", [1, D], FP32, kind="ExternalInput")
    out = nc.dram_tensor("out", [NQ, D], FP32, kind="ExternalOutput")

    with tile.TileContext(nc) as tc:
        with (
            tc.tile_pool(name="perm", bufs=1) as perm,
            tc.tile_pool(name="ps", bufs=1, space="PSUM") as ps,
        ):
            gamma_t = perm.tile([128, D], FP32)
            beta_t = perm.tile([128, D], FP32)
            nc.gpsimd.dma_start(out=gamma_t, in_=_broadcast_ap(gamma[0:1, :], 128))
            nc.gpsimd.dma_start(out=beta_t, in_=_broadcast_ap(beta[0:1, :], 128))
            eps_t = perm.tile([128, 1], FP32)
            nc.vector.memset(eps_t, 1e-5)
            ones_f = perm.tile([128, 64], FP32)
            nc.vector.memset(ones_f, 1.0)
            ones_t = perm.tile([128, 64], F32R)
            nc.vector.tensor_copy(ones_t, ones_f)

            wq_t = perm.tile([128, 8, D], BF16)
            wo_t = perm.tile([128, 8, D], BF16)
            nc.sync.dma_start(wq_t, wq[:, :].rearrange("(dt p) f -> p dt f", p=128))
            nc.sync.dma_start(wo_t, wo[:, :].rearrange("(dt p) f -> p dt f", p=128))

            # ksT[jt][p, m] = ks[m, jt*128+p]; vsp[kt][p, 65h:65h+64] = vs[kt*128+p, 64h:64h+64], col 65h+64 = 1
            ksT = [perm.tile([128, NK], BF16, tag=f"ks{j}", name=f"ks{j}") for j in range(8)]
            vsp = [perm.tile([128, 16 * 65], BF16, tag=f"vsp{t}", name=f"vsp{t}") for t in range(16)]

            with tc.tile_pool(name="kv", bufs=1) as kv:
                wkv_t = kv.tile([128, 8, D], BF16, tag="wkv")
                nc.sync.dma_start(wkv_t, wv[:, :].rearrange("(dt p) f -> p dt f", p=128))
                for rc in range(8):
                    vstage = kv.tile([128, 8, 256], BF16, tag="stage", bufs=2)
                    nc.sync.dma_start(
                        vstage,
                        vT[:, rc * 256:(rc + 1) * 256].rearrange("(dt p) n -> p dt n", p=128),
                    )
                    for rt in range(2):
                        kt_i = rc * 2 + rt
                        for jc in range(2):
                            pp = ps.tile([128, 512], FP32, tag="pp", bufs=2)
                            for dt_i in range(8):
                                nc.tensor.matmul(
                                    pp,
                                    vstage[:, dt_i, rt * 128:(rt + 1) * 128],
                                    wkv_t[:, dt_i, jc * 512:(jc + 1) * 512],
                                    start=(dt_i == 0),
                                    stop=(dt_i == 7),
                                )
                            nc.vector.tensor_copy(
                                vsp[kt_i].rearrange("p (h c) -> p h c", h=16)[:, jc * 8:(jc + 1) * 8, 0:64],
                                pp.rearrange("p (h c) -> p h c", h=8),
                            )
                        nc.vector.memset(
                            vsp[kt_i].rearrange("p (h c) -> p h c", h=16)[:, :, 64:65], 1.0
                        )

                nc.sync.dma_start(wkv_t, wk[:, :].rearrange("(dt p) f -> p dt f", p=128))
                for kc in range(4):
                    kstage = kv.tile([128, 8, 512], BF16, tag="kstage", bufs=2)
                    nc.sync.dma_start(
                        kstage,
                        kT[:, kc * 512:(kc + 1) * 512].rearrange("(dt p) n -> p dt n", p=128),
                    )
                    for jt in range(8):
                        pp = ps.tile([128, 512], FP32, tag="pp", bufs=2)
                        for dt_i in range(8):
                            nc.tensor.matmul(
                                pp,
                                wkv_t[:, dt_i, jt * 128:(jt + 1) * 128],
                                kstage[:, dt_i, :],
                                start=(dt_i == 0),
                                stop=(dt_i == 7),
                            )
                        nc.vector.tensor_copy(ksT[jt][:, kc * 512:(kc + 1) * 512], pp)

            with tc.tile_pool(name="qp", bufs=1) as qp:
                for qc in range(2):
                    qstage = qp.tile([128, 8, 512], BF16, tag="qstage")
                    nc.sync.dma_start(
                        qstage,
                        qT[:, qc * 512:(qc + 1) * 512].rearrange("(dt p) n -> p dt n", p=128),
                    )
                    qs_t = [qp.tile([128, 512], BF16, tag=f"qs{j}", name=f"qs{j}") for j in range(8)]
                    for jt in range(8):
                        pp = ps.tile([128, 512], FP32, tag="pp", bufs=2)
                        for dt_i in range(8):
                            nc.tensor.matmul(
                                pp,
                                wq_t[:, dt_i, jt * 128:(jt + 1) * 128],
                                qstage[:, dt_i, :],
                                start=(dt_i == 0),
                                stop=(dt_i == 7),
                            )
                        nc.vector.tensor_copy(qs_t[jt], pp)

                    at_t = [qp.tile([128, 512], BF16, tag=f"at{j}", name=f"at{j}") for j in range(8)]
                    pending = [None]

                    def emit_norm(p, at_t=at_t):
                        ues_, uos_, hp_ = p
                        rd1 = qp.tile([128, 512], F32R, tag="rd", bufs=2)
                        with nc.allow_low_precision(reason="f32r recip, ~1.5e-4 rel"):
                            nc.vector.reciprocal(rd1[64:65, :], ues_[64:65, :])
                        bce = ps.tile([128, 512], FP32, tag="pp", bufs=2)
                        nc.tensor.matmul(
                            bce[0:64, :], ones_t[64:65, 0:64], rd1[64:65, :],
                            start=True, stop=True, tile_position=(64, 0),
                        )
                        nc.vector.tensor_mul(at_t[hp_][0:64, :], ues_[0:64, :], bce[0:64, :])
                        rd2 = qp.tile([128, 512], F32R, tag="rd", bufs=2)
                        with nc.allow_low_precision(reason="f32r recip, ~1.5e-4 rel"):
                            nc.vector.reciprocal(rd2[64:65, :], uos_[64:65, :])
                        bco = ps.tile([128, 512], FP32, tag="pp", bufs=2)
                        nc.tensor.matmul(
                            bco[0:64, :], ones_t[64:65, 0:64], rd2[64:65, :],
                            start=True, stop=True, tile_position=(64, 0),
                        )
                        nc.vector.tensor_mul(at_t[hp_][64:128, :], uos_[0:64, :], bco[0:64, :])

                    for hp in range(8):
                        ue = ps.tile([128, 512], FP32, tag="ue")
                        uo = ps.tile([128, 512], FP32, tag="uo")
                        Es = {}

                        def emit_sc(kt, hp=hp):
                            sc = ps.tile([128, 1024], FP32, tag="sc", bufs=2)
                            nc.tensor.matmul(
                                sc[:, 0:512], ksT[hp][0:64, kt * 128:(kt + 1) * 128],
                                qs_t[hp][0:64, :], start=True, stop=True,
                                skip_group_check=True,
                            )
                            nc.tensor.matmul(
                                sc[:, 512:1024], ksT[hp][64:128, kt * 128:(kt + 1) * 128],
                                qs_t[hp][64:128, :], start=True, stop=True,
                                tile_position=(64, 0), skip_group_check=True,
                            )
                            E = qp.tile([128, 1024], BF16, tag="E", bufs=3)
                            nc.scalar.activation(E, sc, func=EXP, bias=0.0, scale=0.125)
                            Es[kt] = E

                        emit_sc(0)
                        for kt in range(16):
                            if kt + 1 < 16:
                                emit_sc(kt + 1)
                            if kt == 2 and pending[0] is not None:
                                emit_norm(pending[0])
                                pending[0] = None
                            Ep = Es.pop(kt)
                            nc.tensor.matmul(
                                ue[0:65, :], vsp[kt][:, 130 * hp:130 * hp + 65],
                                Ep[:, 0:512],
                                start=(kt == 0), stop=(kt == 15), skip_group_check=True,
                            )
                            nc.tensor.matmul(
                                uo[0:65, :], vsp[kt][:, 130 * hp + 65:130 * hp + 130],
                                Ep[:, 512:1024],
                                start=(kt == 0), stop=(kt == 15), skip_group_check=True,
                            )
                        ues = qp.tile([128, 512], FP32, tag="ues", bufs=2)
                        uos = qp.tile([128, 512], FP32, tag="uos", bufs=2)
                        nc.vector.tensor_copy(ues[0:65, :], ue[0:65, :])
                        nc.vector.tensor_copy(uos[0:65, :], uo[0:65, :])
                        pending[0] = (ues, uos, hp)
                    emit_norm(pending[0])
                    pending[0] = None

                    for rt in range(4):
                        row0 = qc * 512 + rt * 128
                        outf = qp.tile([128, D], FP32, tag="outf", bufs=2)
                        nc.sync.dma_start(outf, qn[row0:row0 + 128, :])
                        for oc in range(2):
                            po = ps.tile([128, 512], FP32, tag="pp", bufs=2)
                            for it in range(8):
                                nc.tensor.matmul(
                                    po, at_t[it][:, rt * 128:(rt + 1) * 128],
                                    wo_t[:, it, oc * 512:(oc + 1) * 512],
                                    start=(it == 0), stop=(it == 7),
                                )
                            nc.vector.tensor_add(
                                out=outf[:, oc * 512:(oc + 1) * 512],
                                in0=outf[:, oc * 512:(oc + 1) * 512], in1=po,
                            )
                        bst = qp.tile([128, 2, 6], FP32, tag="bst", bufs=2)
                        mv = qp.tile([128, 2], FP32, tag="mv", bufs=2)
                        for sg in range(2):
                            nc.vector.bn_stats(out=bst[:, sg, :], in_=outf[:, sg * 512:(sg + 1) * 512])
                        nc.vector.bn_aggr(out=mv, in_=bst)
                        nc.scalar.activation(
                            out=mv[:, 1:2], in_=mv[:, 1:2], func=SQRT,
                            bias=eps_t[:, :], scale=1.0,
                        )
                        nc.vector.reciprocal(mv[:, 1:2], mv[:, 1:2])
                        y = qp.tile([128, D], FP32, tag="y")
                        nc.vector.tensor_scalar(
                            out=y, in0=outf, scalar1=mv[:, 0:1], scalar2=mv[:, 1:2],
                            op0=mybir.AluOpType.subtract, op1=mybir.AluOpType.mult,
                        )
                        nc.vector.tensor_mul(y, y, gamma_t)
                        nc.vector.tensor_add(out=y, in0=y, in1=beta_t)
                        nc.sync.dma_start(out[row0:row0 + 128, :], y)
    nc.finalize()
    return nc


def kernel(q, k, v, Wq, Wk, Wv, Wo, gamma, beta, _trace=False):
    global _NC, LAST_EXEC_NS
    if _NC is None:
        _NC = _build()
    wqh = Wq.T.astype(bf16)
    wkh = Wk.T.astype(bf16)
    wvh = Wv.T.astype(bf16)
    woh = Wo.T.astype(bf16)
    g = np.ascontiguousarray(np.asarray(gamma, dtype=np.float32).reshape(1, D))
    bt = np.ascontiguousarray(np.asarray(beta, dtype=np.float32).reshape(1, D))
    in_maps = []
    for c in range(8):
        b, hh = divmod(c, 2)
        qb = q[b, hh * NQ:(hh + 1) * NQ, :]
        in_maps.append({
            "qT": qb.T.astype(bf16),
            "qn": np.ascontiguousarray(qb, dtype=np.float32),
            "kT": k[b].T.astype(bf16),
            "vT": v[b].T.astype(bf16),
            "wq": wqh, "wk": wkh, "wv": wvh, "wo": woh,
            "gamma": g, "beta": bt,
        })
    res = bass_utils.run_bass_kernel_spmd(_NC, in_maps, list(range(8)), trace=_trace)
    LAST_EXEC_NS = getattr(res, "exec_time_ns", None)
    outp = np.empty((B, N, D), np.float32)
    for c in range(8):
        b, hh = divmod(c, 2)
        outp[b, hh * NQ:(hh + 1) * NQ, :] = res.results[c]["out"]
    return outp
